# revision 1
# baseline (speedup 1.0000x reference)
"""GroupHadamardLayer (segment_reduce) Trainium2 kernel.

The reference computes, for arbitrary group_idx:
    gathered = x[:, group_idx]                # [B, 256, 8]
    h = einsum('bng,ng->bn', gathered, gc_w)  # [B, 256]
    h = h * diag_w
    out = h @ fc_w                            # [B, 1]

This is linear in x, so it collapses to out = x @ w with
    w[group_idx[n, g]] += gc_w[n, g] * diag_w[n] * fc_w[n, 0]
(scatter-add — exact for duplicate indices too).

Device kernel: pure memory-bound matvec. x [16384, 2048] f32 (128 MiB) is
sharded by batch across 8 cores (2048 rows / 16 MiB each). Each core
streams its shard in 2 MiB chunks ([128 partitions, 2 row-groups, 2048
cols]). Per 128-row group: an elementwise multiply against the
partition-replicated w (VectorE, 1/4 of tiles on GpSimd to balance load),
then a free-dim accumulate on ScalarE (activation Copy + accum_out) giving
the 128 per-row dot products. All compute hides under the DMA stream.
"""

import os
import sys
from contextlib import ExitStack

sys.path.insert(0, "/opt/trn_rl_repo")

import numpy as np

from concourse import bacc, bass, tile
from concourse.bass_utils import run_bass_kernel_spmd

mybir = bass.mybir
F32 = mybir.dt.float32

B, F = 16384, 2048
N_CORES = 8
ROWS = B // N_CORES  # 2048 rows per core
P = 128
G = 2  # 128-row groups per DMA chunk -> [128, 2*2048] f32 = 2 MiB per dma
N_TILES = ROWS // P  # 16
N_CHUNKS = N_TILES // G  # 8

_NC = None
LAST_RESULT = None  # BassKernelResults of the most recent run (for test.py)


def _build_nc():
    # Bacc (not plain Bass): its finalize() runs generate_event_semaphores,
    # which splits multi-sem waits — TRN2 ISA allows 1 sync wait per inst.
    nc = bacc.Bacc("TRN2", target_bir_lowering=False, debug=False)
    x = nc.dram_tensor("x", [ROWS, F], F32, kind="ExternalInput")
    w = nc.dram_tensor("wrep", [P, F], F32, kind="ExternalInput")
    out = nc.dram_tensor("out", [P, N_TILES], F32, kind="ExternalOutput")

    with tile.TileContext(nc) as tc:
        with (
            tc.tile_pool(name="xp", bufs=4) as xp,
            tc.tile_pool(name="pp", bufs=5) as pp,
            tc.tile_pool(name="wp", bufs=1) as wp,
            tc.tile_pool(name="op", bufs=1) as op,
        ):
            # w arrives host-replicated to all 128 partitions (1 MiB). The
            # alternatives all lose: stride-0 DMA APs and GpSimd
            # partition_broadcast fail on this stack, and a TensorE K=1
            # broadcast (8 KB load + 8 fp32 matmuls + PSUM copy) finishes
            # ~4 us LATER than just streaming the 1 MiB (fp32 matmul is
            # quarter-rate and the cold 8 KB DMA alone takes ~5 us).
            w_t = wp.tile([P, F], F32)
            nc.sync.dma_start(w_t[:], w.ap())
            out_t = op.tile([P, N_TILES], F32)
            dummy = wp.tile([P, 1], F32)

            # Row-group schedule: two 1-rowgroup (1 MiB) chunks first to cut
            # the pipeline-fill latency, then 2-rowgroup (2 MiB) chunks.
            chunk_sizes = [1, 1] + [G] * ((N_TILES - 4) // G) + [1, 1]
            # (wrep streams first on the same ring; a small chunk 0 means the
            # first multiply waits for only wrep + 1 MiB.)
            assert sum(chunk_sizes) == N_TILES
            xr = x.ap().rearrange("(t p) n -> t p n", p=P)  # [16, 128, 2048]
            t = 0
            for size in chunk_sizes:
                x_t = xp.tile([P, G, F], F32, tag="x")
                # chunk covers row-groups [t, t+size)
                src = x.ap()[t * P : (t + size) * P, :].rearrange(
                    "(g p) n -> p g n", p=P
                )
                nc.sync.dma_start(x_t[:, :size, :], src)
                for g in range(size):
                    prod = pp.tile([P, F], F32)
                    # VectorE: prod = x_rowgroup * w. (GpSimd offload was
                    # tried and reverted: its 2-input TT contends for SBUF
                    # ports and slows concurrent DVE TTs 2-3x.)
                    nc.vector.tensor_tensor(
                        out=prod[:],
                        in0=x_t[:, g, :],
                        in1=w_t[:],
                        op=mybir.AluOpType.mult,
                    )
                    # ScalarE: row dot product = sum_free(prod). out is a
                    # stride-0 dummy — only accum_out matters.
                    nc.scalar.activation(
                        out=dummy.broadcast_to((P, F)),
                        in_=prod[:],
                        func=mybir.ActivationFunctionType.Copy,
                        accum_out=out_t[:, t + g : t + g + 1],
                    )
                t += size
                if t == N_TILES // 2:
                    # First half of the outputs: DMA out early so only the
                    # last few rows' results trail the final chunk.
                    nc.sync.dma_start(
                        out.ap()[:, : N_TILES // 2], out_t[:, : N_TILES // 2]
                    )
            nc.sync.dma_start(
                out.ap()[:, N_TILES // 2 :], out_t[:, N_TILES // 2 :]
            )
    nc.finalize()
    return nc


def kernel(x, group_idx, gc_w, diag_w, fc_w):
    global _NC, LAST_RESULT
    x = np.ascontiguousarray(np.asarray(x, dtype=np.float32))
    gi = np.asarray(group_idx).astype(np.int64)
    gc_w = np.asarray(gc_w, dtype=np.float32)
    diag_w = np.asarray(diag_w, dtype=np.float32).reshape(-1)
    fc_w = np.asarray(fc_w, dtype=np.float32).reshape(-1, 1)

    # Fold everything linear into one combined weight vector (exact).
    coef = gc_w * diag_w[:, None] * fc_w  # [256, 8]
    w = np.zeros(F, dtype=np.float32)
    np.add.at(w, gi.ravel(), coef.ravel().astype(np.float32))
    wrep = np.ascontiguousarray(np.broadcast_to(w, (P, F))).astype(np.float32)

    if _NC is None:
        _NC = _build_nc()

    in_maps = [
        {"x": np.ascontiguousarray(x[i * ROWS : (i + 1) * ROWS]), "wrep": wrep}
        for i in range(N_CORES)
    ]
    trace = bool(int(os.environ.get("TRN_KERNEL_TRACE", "0")))
    LAST_RESULT = run_bass_kernel_spmd(
        _NC, in_maps, list(range(N_CORES)), trace=trace
    )
    # out[p, t] is the dot product for shard row t*128 + p
    shard_outs = [
        LAST_RESULT.results[i]["out"].T.reshape(ROWS) for i in range(N_CORES)
    ]
    return np.concatenate(shard_outs).reshape(B, 1).astype(np.float32)



# revision 5
# speedup vs baseline: 1.0198x; 1.0198x over previous
"""GroupHadamardLayer (segment_reduce) Trainium2 kernel — baseline copy.

TT on VectorE + ACTIVATE accumulate on ScalarE, 2 MiB chunks.
"""

import os
import sys
from contextlib import ExitStack

sys.path.insert(0, "/opt/trn_rl_repo")

import numpy as np

from concourse import bacc, bass, tile
from concourse.bass_utils import run_bass_kernel_spmd

mybir = bass.mybir
F32 = mybir.dt.float32

B, F = 16384, 2048
N_CORES = 8
ROWS = B // N_CORES  # 2048 rows per core
P = 128
G = 2  # 128-row groups per DMA chunk -> [128, 2*2048] f32 = 2 MiB per dma
N_TILES = ROWS // P  # 16
N_CHUNKS = N_TILES // G  # 8

_NC = None
LAST_RESULT = None  # BassKernelResults of the most recent run (for test.py)


def _build_nc():
    # Bacc (not plain Bass): its finalize() runs generate_event_semaphores,
    # which splits multi-sem waits — TRN2 ISA allows 1 sync wait per inst.
    nc = bacc.Bacc("TRN2", target_bir_lowering=False, debug=False)
    x = nc.dram_tensor("x", [ROWS, F], F32, kind="ExternalInput")
    w = nc.dram_tensor("wrep", [P, F], F32, kind="ExternalInput")
    out = nc.dram_tensor("out", [P, N_TILES], F32, kind="ExternalOutput")

    with tile.TileContext(nc) as tc:
        with (
            tc.tile_pool(name="xp", bufs=4) as xp,
            tc.tile_pool(name="pp", bufs=5) as pp,
            tc.tile_pool(name="wp", bufs=1) as wp,
            tc.tile_pool(name="op", bufs=1) as op,
        ):
            # w arrives host-replicated to all 128 partitions (1 MiB).
            w_t = wp.tile([P, F], F32)
            nc.sync.dma_start(w_t[:], w.ap())
            out_t = op.tile([P, N_TILES], F32)
            dummy = wp.tile([P, 1], F32)

            # Row-group schedule: two 1-rowgroup (1 MiB) chunks first to cut
            # the pipeline-fill latency, then 2-rowgroup (2 MiB) chunks.
            chunk_sizes = [1, 1] + [G] * ((N_TILES - 4) // G) + [1, 1]
            assert sum(chunk_sizes) == N_TILES
            xr = x.ap().rearrange("(t p) n -> t p n", p=P)  # [16, 128, 2048]
            t = 0
            for size in chunk_sizes:
                x_t = xp.tile([P, G, F], F32, tag="x")
                # chunk covers row-groups [t, t+size)
                src = x.ap()[t * P : (t + size) * P, :].rearrange(
                    "(g p) n -> p g n", p=P
                )
                nc.sync.dma_start(x_t[:, :size, :], src)
                for g in range(size):
                    prod = pp.tile([P, F], F32)
                    nc.vector.tensor_tensor(
                        out=prod[:],
                        in0=x_t[:, g, :],
                        in1=w_t[:],
                        op=mybir.AluOpType.mult,
                    )
                    # ScalarE: row dot product = sum_free(prod). out is a
                    # stride-0 dummy — only accum_out matters.
                    nc.scalar.activation(
                        out=dummy.broadcast_to((P, F)),
                        in_=prod[:],
                        func=mybir.ActivationFunctionType.Copy,
                        accum_out=out_t[:, t + g : t + g + 1],
                    )
                t += size
                if t == N_TILES // 2:
                    nc.sync.dma_start(
                        out.ap()[:, : N_TILES // 2], out_t[:, : N_TILES // 2]
                    )
            nc.sync.dma_start(
                out.ap()[:, N_TILES // 2 :], out_t[:, N_TILES // 2 :]
            )
    nc.finalize()
    return nc


def kernel(x, group_idx, gc_w, diag_w, fc_w):
    global _NC, LAST_RESULT
    x = np.ascontiguousarray(np.asarray(x, dtype=np.float32))
    gi = np.asarray(group_idx).astype(np.int64)
    gc_w = np.asarray(gc_w, dtype=np.float32)
    diag_w = np.asarray(diag_w, dtype=np.float32).reshape(-1)
    fc_w = np.asarray(fc_w, dtype=np.float32).reshape(-1, 1)

    # Fold everything linear into one combined weight vector (exact).
    coef = gc_w * diag_w[:, None] * fc_w  # [256, 8]
    w = np.zeros(F, dtype=np.float32)
    np.add.at(w, gi.ravel(), coef.ravel().astype(np.float32))
    wrep = np.ascontiguousarray(np.broadcast_to(w, (P, F))).astype(np.float32)

    if _NC is None:
        _NC = _build_nc()

    in_maps = [
        {"x": np.ascontiguousarray(x[i * ROWS : (i + 1) * ROWS]), "wrep": wrep}
        for i in range(N_CORES)
    ]
    trace = bool(int(os.environ.get("TRN_KERNEL_TRACE", "0")))
    LAST_RESULT = run_bass_kernel_spmd(
        _NC, in_maps, list(range(N_CORES)), trace=trace
    )
    # out[p, t] is the dot product for shard row t*128 + p
    shard_outs = [
        LAST_RESULT.results[i]["out"].T.reshape(ROWS) for i in range(N_CORES)
    ]
    return np.concatenate(shard_outs).reshape(B, 1).astype(np.float32)


# revision 6
# speedup vs baseline: 1.0970x; 1.0757x over previous
"""GroupHadamardLayer (segment_reduce) Trainium2 kernel.

The reference computes, for arbitrary group_idx:
    gathered = x[:, group_idx]                # [B, 256, 8]
    h = einsum('bng,ng->bn', gathered, gc_w)  # [B, 256]
    h = h * diag_w
    out = h @ fc_w                            # [B, 1]

This is linear in x, so it collapses to out = x @ w with
    w[group_idx[n, g]] += gc_w[n, g] * diag_w[n] * fc_w[n, 0]
(scatter-add — exact for duplicate indices too).

Device kernel: pure memory-bound matvec. x [16384, 2048] f32 (128 MiB) is
sharded by batch across 8 cores (2048 rows / 16 MiB each). Each core
streams its shard as 1 MiB row-group chunks [128, 2048] (contiguous DRAM
reads). Per row-group: VectorE multiply against the partition-replicated
w, then a ScalarE activation-accumulate gives the 128 per-row dot
products. Deep tile pools (8 x-bufs, 6 prod-bufs) keep the HBM stream
free of compute backpressure; the final row-group is split into two
column halves so only ~2.5 us of compute trails the last HBM byte.
"""

import os
import sys
from contextlib import ExitStack

sys.path.insert(0, "/opt/trn_rl_repo")

import numpy as np

from concourse import bacc, bass, tile
from concourse.bass_utils import run_bass_kernel_spmd

mybir = bass.mybir
F32 = mybir.dt.float32

B, F = 16384, 2048
N_CORES = 8
ROWS = B // N_CORES  # 2048 rows per core
P = 128
N_TILES = ROWS // P  # 16 row-groups of 1 MiB each
H = F // 2  # column half for the tail split

# Compute-stage variant, switchable for A/B experiments:
#   tt_act: TT multiply on VectorE + ACTIVATE accumulate on ScalarE (safe)
#   stt:    fused scalar_tensor_tensor w/ accum_out on VectorE
#   ttr:    fused tensor_tensor_reduce on VectorE
VARIANT = os.environ.get("KERNEL_VARIANT", "tt_act")

_NC = None
LAST_RESULT = None  # BassKernelResults of the most recent run (for test.py)


def _build_nc():
    # Bacc (not plain Bass): its finalize() runs generate_event_semaphores,
    # which splits multi-sem waits — TRN2 ISA allows 1 sync wait per inst.
    nc = bacc.Bacc("TRN2", target_bir_lowering=False, debug=False)
    x = nc.dram_tensor("x", [ROWS, F], F32, kind="ExternalInput")
    w = nc.dram_tensor("wrep", [P, F], F32, kind="ExternalInput")
    out = nc.dram_tensor("out", [P, N_TILES], F32, kind="ExternalOutput")

    with tile.TileContext(nc) as tc:
        with (
            tc.tile_pool(name="xp", bufs=8) as xp,
            tc.tile_pool(name="hp", bufs=2) as hp,
            tc.tile_pool(name="pp", bufs=6) as pp,
            tc.tile_pool(name="php", bufs=2) as php,
            tc.tile_pool(name="wp", bufs=1) as wp,
            tc.tile_pool(name="op", bufs=1) as op,
        ):
            # w arrives host-replicated to all 128 partitions (1 MiB). The
            # alternatives all lose: stride-0 DMA APs and GpSimd
            # partition_broadcast fail on this stack, and a TensorE K=1
            # broadcast (8 KB load + 8 fp32 matmuls + PSUM copy) finishes
            # ~4 us LATER than just streaming the 1 MiB (fp32 matmul is
            # quarter-rate and the cold 8 KB DMA alone takes ~5 us).
            w_t = wp.tile([P, F], F32)
            nc.sync.dma_start(w_t[:], w.ap())
            out_t = op.tile([P, N_TILES], F32)
            pa = op.tile([P, 2], F32)  # partial accums of the split tail
            dummy = wp.tile([P, 1], F32)

            def reduce_rowgroup(x_ap, w_ap, acc_ap, ncols):
                """acc_ap[p, 0] = sum_f x_ap[p, f] * w_ap[p, f]."""
                if VARIANT == "stt":
                    scratch = pp.tile([P, F], F32, tag="prod")
                    nc.vector.scalar_tensor_tensor(
                        out=scratch[:, :ncols],
                        in0=x_ap,
                        scalar=1.0,
                        in1=w_ap,
                        op0=mybir.AluOpType.mult,
                        op1=mybir.AluOpType.mult,
                        accum_out=acc_ap,
                    )
                elif VARIANT == "ttr":
                    scratch = pp.tile([P, F], F32, tag="prod")
                    nc.vector.tensor_tensor_reduce(
                        out=scratch[:, :ncols],
                        in0=x_ap,
                        in1=w_ap,
                        scale=1.0,
                        scalar=0.0,
                        op0=mybir.AluOpType.mult,
                        op1=mybir.AluOpType.add,
                        accum_out=acc_ap,
                    )
                else:  # tt_act
                    pool = pp if ncols == F else php
                    prod = pool.tile([P, ncols], F32, tag="prod")
                    nc.vector.tensor_tensor(
                        out=prod[:],
                        in0=x_ap,
                        in1=w_ap,
                        op=mybir.AluOpType.mult,
                    )
                    # ScalarE: dot product = sum_free(prod). out is a
                    # stride-0 dummy — only accum_out matters.
                    nc.scalar.activation(
                        out=dummy.broadcast_to((P, ncols)),
                        in_=prod[:],
                        func=mybir.ActivationFunctionType.Copy,
                        accum_out=acc_ap,
                    )

            # Row-groups 0..14: full-width 1 MiB chunks.
            for t in range(N_TILES - 1):
                x_t = xp.tile([P, F], F32, tag="x")
                nc.sync.dma_start(x_t[:], x.ap()[t * P : (t + 1) * P, :])
                reduce_rowgroup(x_t[:], w_t[:], out_t[:, t : t + 1], F)
                if t == N_TILES // 2 - 1:
                    # First half of the outputs: DMA out early so only the
                    # last few rows' results trail the final chunk.
                    nc.sync.dma_start(
                        out.ap()[:, : N_TILES // 2], out_t[:, : N_TILES // 2]
                    )

            # Row-group 15: two column halves so the multiply of half a
            # overlaps the DMA of half b — only half-width compute trails
            # the final HBM byte.
            t = N_TILES - 1
            for h in range(2):
                x_h = hp.tile([P, H], F32, tag="xh")
                nc.sync.dma_start(
                    x_h[:], x.ap()[t * P : (t + 1) * P, h * H : (h + 1) * H]
                )
                reduce_rowgroup(
                    x_h[:], w_t[:, h * H : (h + 1) * H], pa[:, h : h + 1], H
                )
            # Combine the two halves' partial sums.
            nc.vector.tensor_tensor(
                out=out_t[:, t : t + 1],
                in0=pa[:, 0:1],
                in1=pa[:, 1:2],
                op=mybir.AluOpType.add,
            )
            nc.sync.dma_start(
                out.ap()[:, N_TILES // 2 :], out_t[:, N_TILES // 2 :]
            )
    nc.finalize()
    return nc


def kernel(x, group_idx, gc_w, diag_w, fc_w):
    global _NC, LAST_RESULT
    x = np.ascontiguousarray(np.asarray(x, dtype=np.float32))
    gi = np.asarray(group_idx).astype(np.int64)
    gc_w = np.asarray(gc_w, dtype=np.float32)
    diag_w = np.asarray(diag_w, dtype=np.float32).reshape(-1)
    fc_w = np.asarray(fc_w, dtype=np.float32).reshape(-1, 1)

    # Fold everything linear into one combined weight vector (exact).
    coef = gc_w * diag_w[:, None] * fc_w  # [256, 8]
    w = np.zeros(F, dtype=np.float32)
    np.add.at(w, gi.ravel(), coef.ravel().astype(np.float32))
    wrep = np.ascontiguousarray(np.broadcast_to(w, (P, F))).astype(np.float32)

    if _NC is None:
        _NC = _build_nc()

    in_maps = [
        {"x": np.ascontiguousarray(x[i * ROWS : (i + 1) * ROWS]), "wrep": wrep}
        for i in range(N_CORES)
    ]
    trace = bool(int(os.environ.get("TRN_KERNEL_TRACE", "0")))
    LAST_RESULT = run_bass_kernel_spmd(
        _NC, in_maps, list(range(N_CORES)), trace=trace
    )
    # out[p, t] is the dot product for shard row t*128 + p
    shard_outs = [
        LAST_RESULT.results[i]["out"].T.reshape(ROWS) for i in range(N_CORES)
    ]
    return np.concatenate(shard_outs).reshape(B, 1).astype(np.float32)


# revision 7
# speedup vs baseline: 1.1086x; 1.0106x over previous
"""GroupHadamardLayer (segment_reduce) Trainium2 kernel.

The reference computes, for arbitrary group_idx:
    gathered = x[:, group_idx]                # [B, 256, 8]
    h = einsum('bng,ng->bn', gathered, gc_w)  # [B, 256]
    h = h * diag_w
    out = h @ fc_w                            # [B, 1]

This is linear in x, so it collapses to out = x @ w with
    w[group_idx[n, g]] += gc_w[n, g] * diag_w[n] * fc_w[n, 0]
(scatter-add — exact for duplicate indices too).

Device kernel: pure memory-bound matvec. x [16384, 2048] f32 (128 MiB) is
sharded by batch across 8 cores (2048 rows / 16 MiB each). Each core
streams its shard as 16 contiguous 1 MiB row-group chunks [128, 2048].
Per row-group the dot products against the partition-replicated w are
reduced on-chip; deep tile pools keep the HBM stream free of compute
backpressure (the chunk-dispatch instruction on SyncE carries the
buffer-reuse wait, so shallow pools stall the DMA ring itself).
exec_time ~= last-flush-dispatch + 4.3 us (fixed epilogue), so the
kernel minimizes trailing compute after the final HBM byte.
"""

import os
import sys
from contextlib import ExitStack

sys.path.insert(0, "/opt/trn_rl_repo")

import numpy as np

from concourse import bacc, bass, tile
from concourse.bass_utils import run_bass_kernel_spmd

mybir = bass.mybir
F32 = mybir.dt.float32

B, F = 16384, 2048
N_CORES = 8
ROWS = B // N_CORES  # 2048 rows per core
P = 128
N_TILES = ROWS // P  # 16 row-groups of 1 MiB each

# Compute-stage variant, switchable for A/B experiments:
#   tt_act: TT multiply on VectorE + ACTIVATE accumulate on ScalarE (safe)
#   stt:    fused scalar_tensor_tensor w/ accum_out on VectorE
#   ttr:    fused tensor_tensor_reduce on VectorE
VARIANT = os.environ.get("KERNEL_VARIANT", "tt_act")

_NC = None
LAST_RESULT = None  # BassKernelResults of the most recent run (for test.py)


def _build_nc():
    # Bacc (not plain Bass): its finalize() runs generate_event_semaphores,
    # which splits multi-sem waits — TRN2 ISA allows 1 sync wait per inst.
    nc = bacc.Bacc("TRN2", target_bir_lowering=False, debug=False)
    x = nc.dram_tensor("x", [ROWS, F], F32, kind="ExternalInput")
    w = nc.dram_tensor("wrep", [P, F], F32, kind="ExternalInput")
    out = nc.dram_tensor("out", [P, N_TILES], F32, kind="ExternalOutput")

    with tile.TileContext(nc) as tc:
        with (
            tc.tile_pool(name="xp", bufs=10) as xp,
            tc.tile_pool(name="pp", bufs=8) as pp,
            tc.tile_pool(name="wp", bufs=1) as wp,
            tc.tile_pool(name="op", bufs=1) as op,
        ):
            # w arrives host-replicated to all 128 partitions (1 MiB). The
            # alternatives all lose: stride-0 DMA APs and GpSimd
            # partition_broadcast fail on this stack, and a TensorE K=1
            # broadcast (8 KB load + 8 fp32 matmuls + PSUM copy) finishes
            # ~4 us LATER than just streaming the 1 MiB (fp32 matmul is
            # quarter-rate and the cold 8 KB DMA alone takes ~5 us).
            w_t = wp.tile([P, F], F32)
            nc.sync.dma_start(w_t[:], w.ap())
            out_t = op.tile([P, N_TILES], F32)
            dummy = wp.tile([P, 1], F32)

            def reduce_rowgroup(x_ap, acc_ap):
                """acc_ap[p, 0] = sum_f x_ap[p, f] * w_t[p, f]."""
                if VARIANT == "stt":
                    nc.vector.scalar_tensor_tensor(
                        out=dummy.broadcast_to((P, F)),
                        in0=x_ap,
                        scalar=1.0,
                        in1=w_t[:],
                        op0=mybir.AluOpType.mult,
                        op1=mybir.AluOpType.mult,
                        accum_out=acc_ap,
                    )
                elif VARIANT == "ttr":
                    nc.vector.tensor_tensor_reduce(
                        out=dummy.broadcast_to((P, F)),
                        in0=x_ap,
                        in1=w_t[:],
                        scale=1.0,
                        scalar=0.0,
                        op0=mybir.AluOpType.mult,
                        op1=mybir.AluOpType.add,
                        accum_out=acc_ap,
                    )
                else:  # tt_act
                    prod = pp.tile([P, F], F32, tag="prod")
                    nc.vector.tensor_tensor(
                        out=prod[:],
                        in0=x_ap,
                        in1=w_t[:],
                        op=mybir.AluOpType.mult,
                    )
                    # ScalarE: dot product = sum_free(prod). out is a
                    # stride-0 dummy — only accum_out matters.
                    nc.scalar.activation(
                        out=dummy.broadcast_to((P, F)),
                        in_=prod[:],
                        func=mybir.ActivationFunctionType.Copy,
                        accum_out=acc_ap,
                    )

            for t in range(N_TILES):
                x_t = xp.tile([P, F], F32, tag="x")
                # rows [t*128, (t+1)*128): contiguous 1 MiB DRAM read
                nc.sync.dma_start(x_t[:], x.ap()[t * P : (t + 1) * P, :])
                reduce_rowgroup(x_t[:], out_t[:, t : t + 1])
                if t == N_TILES // 2 - 1:
                    # First half of the outputs: DMA out early so only the
                    # last rows' results trail the final chunk.
                    nc.sync.dma_start(
                        out.ap()[:, : N_TILES // 2], out_t[:, : N_TILES // 2]
                    )
            nc.sync.dma_start(
                out.ap()[:, N_TILES // 2 :], out_t[:, N_TILES // 2 :]
            )
    nc.finalize()
    return nc


def kernel(x, group_idx, gc_w, diag_w, fc_w):
    global _NC, LAST_RESULT
    x = np.ascontiguousarray(np.asarray(x, dtype=np.float32))
    gi = np.asarray(group_idx).astype(np.int64)
    gc_w = np.asarray(gc_w, dtype=np.float32)
    diag_w = np.asarray(diag_w, dtype=np.float32).reshape(-1)
    fc_w = np.asarray(fc_w, dtype=np.float32).reshape(-1, 1)

    # Fold everything linear into one combined weight vector (exact).
    coef = gc_w * diag_w[:, None] * fc_w  # [256, 8]
    w = np.zeros(F, dtype=np.float32)
    np.add.at(w, gi.ravel(), coef.ravel().astype(np.float32))
    wrep = np.ascontiguousarray(np.broadcast_to(w, (P, F))).astype(np.float32)

    if _NC is None:
        _NC = _build_nc()

    in_maps = [
        {"x": np.ascontiguousarray(x[i * ROWS : (i + 1) * ROWS]), "wrep": wrep}
        for i in range(N_CORES)
    ]
    trace = bool(int(os.environ.get("TRN_KERNEL_TRACE", "0")))
    LAST_RESULT = run_bass_kernel_spmd(
        _NC, in_maps, list(range(N_CORES)), trace=trace
    )
    # out[p, t] is the dot product for shard row t*128 + p
    shard_outs = [
        LAST_RESULT.results[i]["out"].T.reshape(ROWS) for i in range(N_CORES)
    ]
    return np.concatenate(shard_outs).reshape(B, 1).astype(np.float32)
